# revision 13
# baseline (speedup 1.0000x reference)
"""GRU cell kernel for Trainium2, data-parallel across 8 NeuronCores.

Per core: batch shard of 1024 rows; weights replicated.
  u  = sigmoid(x @ Wxu + h @ Whu + bu)
  r  = sigmoid(x @ Wxr + h @ Whr + br)
  c' = tanh  (x @ Wxc + (h*r) @ Whc + bc)
  c  = u*c' + (1-u)*h  =  u*c' - (u-1)*h

All layout work happens on the HOST: activations are fed pre-transposed
([feature, batch]) and pre-packed in the exact SBUF column order, weights
pre-converted to bf16 in stationary-operand order, and the output comes
back feature-major/bf16 and is unscrambled host-side.  The device runs
nothing but the 768 bf16 matmuls (128x128x512 each), the activations and
the blend -- no on-chip transposes, no dtype casts.

Gate r's first batch-half is phase-split (all 8 output chunks' x-side
matmuls first, h-sides after) so the PE has dense work while the h/Whr
DMAs stream in.  (u-1)*h is precomputed during the u phase so the
candidate tail is only tanh -> mul -> sub -> DMA.

SBUF column layouts (all tiles [128 partitions x cols]):
  activations x,h : col = n*4096 + k*512 + b   (n: batch half, k: feat chunk)
  weights         : col = j*1024 + k*128 + jc  (j: out chunk, k: contraction)
  output          : col = j*1024 + n*512 + b
"""

import os
import sys

import numpy as np

B = 8192
E = 1024
H = 1024
NCORES = 8
B_SH = B // NCORES  # 1024 rows per core

P = 128
KE = E // P   # 8 contraction chunks
NJ = H // P   # 8 output feature chunks
BN = 512      # moving free-dim per matmul
NB = B_SH // BN  # 2

W_NAMES = ("Wxu", "Whu", "Wxr", "Whr", "Wxc", "Whc")
B_NAMES = ("bu", "br", "bc")

_NC_CACHE = {}


def _ensure_paths():
    for p in ("/opt/trn_rl_repo", "/root/.axon_site/_ro/trn_rl_repo"):
        if os.path.isdir(p) and p not in sys.path:
            sys.path.insert(0, p)


def _bf16():
    import ml_dtypes

    return ml_dtypes.bfloat16


def pack_act(a):
    """[B_SH, 1024] f32 -> [128, 8192] bf16, col = n*4096 + k*512 + b."""
    t = np.asarray(a, np.float32).astype(_bf16()).T          # [feat, batch]
    t = t.reshape(KE, P, NB, BN)                             # [k, p, n, b]
    return np.ascontiguousarray(t.transpose(1, 2, 0, 3).reshape(P, NB * KE * BN))


def pack_w(w):
    """[1024, 1024] f32 -> [128, 8192] bf16, col = j*1024 + k*128 + jc."""
    t = np.asarray(w, np.float32).astype(_bf16()).reshape(KE, P, NJ, P)
    return np.ascontiguousarray(t.transpose(1, 2, 0, 3).reshape(P, NJ * KE * P))


def pack_bias(b):
    """[1, 1024] f32 -> [128, 8] f32, col j holds features j*128..j*128+127."""
    return np.ascontiguousarray(
        np.asarray(b, np.float32).reshape(NJ, P).T.astype(np.float32)
    )


def decode_out(o):
    """[128, 8192] bf16 (p; j,n,b cols) -> [B_SH, 1024] f32 [batch, feature]."""
    return np.ascontiguousarray(
        np.asarray(o)
        .astype(np.float32)
        .reshape(P, NJ, NB, BN)
        .transpose(2, 3, 1, 0)
        .reshape(B_SH, H)
    )


def build_in_maps(inputs):
    x = np.asarray(inputs["input"], dtype=np.float32)
    h = np.asarray(inputs["hidden_state"], dtype=np.float32)
    shared = {n: pack_w(inputs[n]) for n in W_NAMES}
    shared.update({n: pack_bias(inputs[n]) for n in B_NAMES})
    in_maps = []
    for c in range(NCORES):
        m = {
            "input": pack_act(x[c * B_SH : (c + 1) * B_SH]),
            "hidden_state": pack_act(h[c * B_SH : (c + 1) * B_SH]),
        }
        m.update(shared)
        in_maps.append(m)
    return in_maps


def _build_nc():
    import concourse.bass as bass
    import concourse.mybir as mybir
    from concourse.tile import TileContext

    f32 = mybir.dt.float32
    bf16 = mybir.dt.bfloat16
    AF = mybir.ActivationFunctionType
    ALU = mybir.AluOpType

    ACT_COLS = NB * KE * BN  # 8192
    W_COLS = NJ * KE * P     # 8192

    nc = bass.Bass()
    x_d = nc.dram_tensor("input", [P, ACT_COLS], bf16, kind="ExternalInput")
    h_d = nc.dram_tensor("hidden_state", [P, ACT_COLS], bf16, kind="ExternalInput")
    w_d = {n: nc.dram_tensor(n, [P, W_COLS], bf16, kind="ExternalInput") for n in W_NAMES}
    b_d = {n: nc.dram_tensor(n, [P, NJ], f32, kind="ExternalInput") for n in B_NAMES}
    out_d = nc.dram_tensor("output", [P, NJ * B_SH], bf16, kind="ExternalOutput")

    with TileContext(nc) as tc:
        with (
            tc.tile_pool(name="sb", bufs=1) as sb,
            tc.tile_pool(name="psum", bufs=1, space="PSUM") as pp,
        ):
            xs = sb.tile([P, ACT_COLS], bf16, tag="xs", bufs=1)
            hs = sb.tile([P, ACT_COLS], bf16, tag="hs", bufs=1)
            us = sb.tile([P, ACT_COLS], bf16, tag="us", bufs=1)
            up = sb.tile([P, ACT_COLS], bf16, tag="up", bufs=1)  # (u-1)*h
            rhs = sb.tile([P, ACT_COLS], bf16, tag="rhs", bufs=1)
            ws = {
                n: sb.tile([P, W_COLS], bf16, tag=f"w_{n}", name=f"w_{n}", bufs=1)
                for n in W_NAMES
            }
            bt = {
                n: sb.tile([P, NJ], f32, tag=f"b_{n}", name=f"b_{n}", bufs=1)
                for n in B_NAMES
            }

            Q = ACT_COLS // 4  # 2048
            HALF = ACT_COLS // 2

            # ---- input DMAs ordered to feed gate r's phase-split start
            JB = KE * P  # 1024 cols = one weight j-block
            nc.sync.dma_start(ws["Wxr"][:, 0:JB], w_d["Wxr"][:, 0:JB])      # j0
            nc.sync.dma_start(xs[:, 0:BN], x_d[:, 0:BN])                    # n0 k0
            nc.sync.dma_start(ws["Wxr"][:, JB:Q], w_d["Wxr"][:, JB:Q])      # j1
            nc.sync.dma_start(xs[:, BN:Q], x_d[:, BN:Q])                    # n0 k1-3
            nc.sync.dma_start(ws["Wxr"][:, Q:HALF], w_d["Wxr"][:, Q:HALF])  # j2,j3
            nc.sync.dma_start(xs[:, Q:HALF], x_d[:, Q:HALF])                # n0 k4-7
            nc.sync.dma_start(ws["Wxr"][:, HALF:], w_d["Wxr"][:, HALF:])    # j4-7
            nc.sync.dma_start(ws["Whr"][:, 0:HALF], w_d["Whr"][:, 0:HALF])  # j0-3
            nc.sync.dma_start(hs[:, 0:HALF], h_d[:, 0:HALF])                # n0
            nc.sync.dma_start(ws["Whr"][:, HALF:], w_d["Whr"][:, HALF:])    # j4-7
            nc.sync.dma_start(xs[:, HALF:], x_d[:, HALF:])                  # n1
            nc.sync.dma_start(hs[:, HALF:], h_d[:, HALF:])                  # n1
            for n in B_NAMES:
                nc.sync.dma_start(bt[n][:, :], b_d[n][:, :])
            for wn in ("Wxu", "Whu", "Wxc", "Whc"):
                nc.sync.dma_start(ws[wn][:, 0:HALF], w_d[wn][:, 0:HALF])
                nc.sync.dma_start(ws[wn][:, HALF:], w_d[wn][:, HALF:])

            def xside(ps, wx, j, n, stop=False):
                for k in range(KE):
                    nc.tensor.matmul(
                        ps[:],
                        wx[:, j * (KE * P) + k * P : j * (KE * P) + (k + 1) * P],
                        xs[:, n * (KE * BN) + k * BN : n * (KE * BN) + (k + 1) * BN],
                        start=(k == 0),
                        stop=False,
                    )

            def hside(ps, wh, rhsT, j, n):
                for k in range(KE):
                    nc.tensor.matmul(
                        ps[:],
                        wh[:, j * (KE * P) + k * P : j * (KE * P) + (k + 1) * P],
                        rhsT[:, n * (KE * BN) + k * BN : n * (KE * BN) + (k + 1) * BN],
                        start=False,
                        stop=(k == KE - 1),
                    )

            def act_slice(t, j, n):
                return t[:, n * (KE * BN) + j * BN : n * (KE * BN) + (j + 1) * BN]

            # ---- PE warmup: dummy matmuls on scratch data during the DMA
            # ramp so the HAM clock-gate releases (1.2->2.4 GHz) before the
            # real matmul stream begins.  Results are never read.
            warm_src = sb.tile([P, BN], bf16, tag="warm_src", bufs=1)
            nc.vector.memset(warm_src[:], 0.0)
            warm_ps = pp.tile([P, BN], f32, tag="mm", name="warm_ps", bufs=8)
            NWARM = 6
            for wi in range(NWARM):
                nc.tensor.matmul(
                    warm_ps[:],
                    warm_src[:, 0:P],
                    warm_src[:],
                    start=(wi == 0),
                    stop=(wi == NWARM - 1),
                )

            def fold_r(ps, j, n):
                rt = sb.tile([P, BN], bf16, tag="rt", bufs=4)
                nc.scalar.activation(
                    rt[:], ps[:], AF.Sigmoid, bias=bt["br"][:, j : j + 1]
                )
                nc.vector.tensor_mul(act_slice(rhs, j, n), rt[:], act_slice(hs, j, n))

            # ---- gate r, n=0: phase-split so PE works while h/Whr stream in
            ps_r0 = [
                pp.tile([P, BN], f32, tag="mm", name=f"ps_r0_{j}", bufs=8)
                for j in range(NJ)
            ]
            for j in range(NJ):
                xside(ps_r0[j], ws["Wxr"], j, 0)
            for j in range(NJ):
                hside(ps_r0[j], ws["Whr"], hs, j, 0)
                fold_r(ps_r0[j], j, 0)

            # ---- gate r, n=1: contiguous groups
            for j in range(NJ):
                ps = pp.tile([P, BN], f32, tag="mm", bufs=8)
                xside(ps, ws["Wxr"], j, 1)
                hside(ps, ws["Whr"], hs, j, 1)
                fold_r(ps, j, 1)

            def paired_group(wx, wh, rhsT, j):
                """Both batch halves accumulated together, k-major, so each
                stationary weight slice is loaded once and used twice."""
                ps = [
                    pp.tile([P, BN], f32, tag="mm", name=f"pg{j}_{n}", bufs=8)
                    for n in range(NB)
                ]
                for k in range(KE):
                    for n in range(NB):
                        nc.tensor.matmul(
                            ps[n][:],
                            wx[:, j * (KE * P) + k * P : j * (KE * P) + (k + 1) * P],
                            xs[:, n * (KE * BN) + k * BN : n * (KE * BN) + (k + 1) * BN],
                            start=(k == 0),
                            stop=False,
                        )
                for k in range(KE):
                    for n in range(NB):
                        nc.tensor.matmul(
                            ps[n][:],
                            wh[:, j * (KE * P) + k * P : j * (KE * P) + (k + 1) * P],
                            rhsT[:, n * (KE * BN) + k * BN : n * (KE * BN) + (k + 1) * BN],
                            start=False,
                            stop=(k == KE - 1),
                        )
                return ps

            # ---- gate u: sigmoid -> us, and up = (u-1)*h for the blend
            for j in range(NJ):
                ps = paired_group(ws["Wxu"], ws["Whu"], hs, j)
                for n in range(NB):
                    nc.scalar.activation(
                        act_slice(us, j, n), ps[n][:], AF.Sigmoid,
                        bias=bt["bu"][:, j : j + 1],
                    )
                    nc.vector.scalar_tensor_tensor(
                        act_slice(up, j, n),
                        act_slice(us, j, n),
                        1.0,
                        act_slice(hs, j, n),
                        ALU.subtract,
                        ALU.mult,
                    )

            # ---- candidate + blend + store:  c = u*c' - (u-1)*h
            # The final group's post-matmul chain is the kernel tail; split it
            # into quarters so ACT/DVE/DMA pipeline instead of serializing.
            for j in range(NJ):
                ps = paired_group(ws["Wxc"], ws["Whc"], rhs, j)
                for n in range(NB):
                    cc = sb.tile([P, BN], bf16, tag="cc", bufs=4)
                    nc.scalar.activation(
                        cc[:], ps[n][:], AF.Tanh, bias=bt["bc"][:, j : j + 1]
                    )
                    nc.vector.tensor_mul(cc[:], cc[:], act_slice(us, j, n))
                    nc.vector.tensor_sub(cc[:], cc[:], act_slice(up, j, n))
                    nc.gpsimd.dma_start(
                        out_d[:, j * B_SH + n * BN : j * B_SH + (n + 1) * BN], cc[:]
                    )

    _split_matmul_waits(nc, mybir)
    return nc


def _split_matmul_waits(nc, mybir):
    """Walrus codegen allows only one sync-wait on a Matmult (it lowers to an
    LDW+MM pair).  Spill extra waits onto a PE NoOp placed just before."""
    n_fixed = 0
    blocks = list(nc.m.functions[0].blocks)
    origs = [list(b.instructions) for b in blocks]
    spill_nops = {}  # id(inst) -> [nop insts]
    for orig in origs:
        for inst in orig:
            si = inst.sync_info
            if (
                si is not None
                and si.on_wait
                and len(si.on_wait) > 1
            ):
                waits = list(si.on_wait)
                eng = nc.engines[inst.engine]
                nops = []
                for w in waits[:-1]:
                    nop = eng.nop(hint="waitspill").ins
                    nop.sync_info = mybir.SyncInfo(on_wait=[w], on_update=[])
                    nops.append(nop)
                inst.sync_info = mybir.SyncInfo(
                    on_wait=waits[-1:], on_update=list(si.on_update or [])
                )
                spill_nops[id(inst)] = nops
                n_fixed += 1
    for blk, orig in zip(blocks, origs):
        new_list = []
        for inst in orig:
            if id(inst) in spill_nops:
                new_list.extend(spill_nops[id(inst)])
            new_list.append(inst)
        # rebuilding from `orig` also drops any freshly created nops that
        # bass appended to this block's tail
        blk.instructions[:] = new_list
    return n_fixed


def get_nc():
    if "nc" not in _NC_CACHE:
        _ensure_paths()
        _NC_CACHE["nc"] = _build_nc()
    return _NC_CACHE["nc"]


def kernel(**inputs):
    _ensure_paths()
    from concourse.bass_utils import run_bass_kernel_spmd

    nc = get_nc()
    in_maps = build_in_maps(inputs)
    res = run_bass_kernel_spmd(nc, in_maps, list(range(NCORES)))
    out = np.concatenate(
        [decode_out(res.results[c]["output"]) for c in range(NCORES)], axis=0
    )
    return out.astype(np.float32)
